# revision 2
# baseline (speedup 1.0000x reference)
"""Trainium2 Bass kernel v2 for the DissipativeRINN problem.

Structure (pure data parallel over batch, 8 cores x 256 batch each,
transposed on-chip layout [feature, batch]):
  - 2-step ROUNDS: round rr handles steps t0=2rr, t0+1.  Per round,
    THREE solve tanh columns per step (i0 wavefront + 2 refinements):
      T0  = tanh [128,512]  : i0(t0+2), i0(t0+3)   (next round's warm
            iterates; stale-x = x(t0), warm-diag from i0(t0), i0(t0+1))
      T12 = tanh [128,1024] : [i1(t0), i1(t0+1) | i2(t0-2), i2(t0-1)]
    The controller output u(t) uses the Anderson-corrected iterate
      w* ~= i2 + (i2 - i1) @ Mc,  Mc = mu Dvw (I - mu Dvw)^-1,  mu=0.7
    folded into two stationary matrices UW2/UW3n (one extra matmul, no
    extra tanh).  Emulator rel err vs reference: 6.5e-3 (gate 2e-2).
  - x-chain (RK4 state update, matrices folded host-side as in v1) rides
    on i0 only, off the tanh critical cycle; x carried bf16 in a 3-slot
    XY ring [48, 512/slot] (x rows 0:16, y rows 32:48, y DMA'd direct).
  - Small-matmul fusions: [U_XY | XND_XY] share one stationary (output
    partition-stacked 0:8 u / 32:48 xnd in one PSUM bank with parity
    slots at base 0/64); value MLP uses block-diagonal W1/W2 (2 token
    blocks per matmul) and partition-stacked W3 ([128,2]).
  - PSUM budget exactly 8 banks: 2x[128,512] i0, 2x[128,1024] T12,
    1 shared xnd/u bank, 1 MLP bank.
"""

import os

import numpy as np
import ml_dtypes

bf16 = ml_dtypes.bfloat16

DT = 0.01
B, T, IN, ST, NL, OUT, H = 2048, 32, 16, 16, 128, 8, 64
NCORES = 8
BL = B // NCORES      # 256
NR = T // 2           # 16 rounds
NG = T // 4           # 8 MLP groups
COLD = int(os.environ.get("K_COLD", "4"))
MU = float(os.environ.get("K_MU", "0.7"))
T_STEPS = T           # test.py compatibility
BB_B3 = 0.0


# ---------------------------------------------------------------------------
# host-side folding (identical math to v1) + v2 matrices
# ---------------------------------------------------------------------------

def _dadd(*ds):
    out = {}
    for d in ds:
        for k, v in d.items():
            out[k] = out.get(k, 0) + v
    return out


def _dmul(d, M):
    return {k: v @ M for k, v in d.items()}


def _dscale(d, s):
    return {k: s * v for k, v in d.items()}


def fold_matrices(inp):
    f64 = lambda k: np.asarray(inp[k], np.float64)
    A_T, Bw_T, By_T = f64("A_T"), f64("Bw_T"), f64("By_T")
    Cv_T, Dvw_T, Dvy_T = f64("Cv_T"), f64("Dvw_T"), f64("Dvy_T")
    Cu_T, Duw_T, Duy_T = f64("Cu_T"), f64("Duw_T"), f64("Duy_T")
    I16 = np.eye(16)
    Z16 = np.zeros((16, 16))
    X = {"XY": np.vstack([I16, Z16])}
    Y = {"XY": np.vstack([Z16, I16])}

    def K_of(Xd, s):
        return _dadd(_dmul(Xd, A_T), _dmul(Y, By_T), {f"W{s}": Bw_T})

    K1 = K_of(X, 1)
    X2 = _dadd(X, _dscale(K1, DT / 2))
    K2 = K_of(X2, 2)
    X3 = _dadd(X, _dscale(K2, DT / 2))
    K3 = K_of(X3, 3)
    X4 = _dadd(X, _dscale(K3, DT))
    K4 = K_of(X4, 4)
    XND = _dscale(_dadd(K1, _dscale(K2, 2), _dscale(K3, 2), K4), DT / 6)
    C1 = _dadd(_dmul(X, Cv_T), _dmul(Y, Dvy_T))["XY"]
    U = _dadd({"XY": np.vstack([Cu_T, Duy_T])}, {"W1": Duw_T})
    return {"C1": C1, "XND": XND, "U": U, "Dvw": Dvw_T}


def pad48(M):
    M = np.asarray(M)
    out = np.zeros((48, M.shape[1]))
    out[0:ST] = M[0:ST]
    out[32:48] = M[ST:2 * ST]
    return out


def build_mats(inp):
    mats = fold_matrices(inp)
    Dvw64 = mats["Dvw"]
    XND_W64 = sum(mats["XND"][f"W{s}"] for s in range(1, 5))
    Mc = (MU * Dvw64) @ np.linalg.inv(np.eye(NL) - MU * Dvw64)
    UW2 = (np.eye(NL) + Mc) @ mats["U"]["W1"]
    UW3n = -(Mc @ mats["U"]["W1"])
    XNDU = np.zeros((48, 48))
    XNDU[:, 0:8] = pad48(mats["U"]["XY"])
    XNDU[:, 32:48] = pad48(mats["XND"]["XY"])
    W1 = np.asarray(inp["W1"], np.float64)
    W2 = np.asarray(inp["W2"], np.float64)
    W3 = np.asarray(inp["W3"], np.float64)
    W1blk = np.zeros((32, 128))
    W1blk[0:16, 0:64] = W1
    W1blk[16:32, 64:128] = W1
    W2blk = np.zeros((128, 128))
    W2blk[0:64, 0:64] = W2
    W2blk[64:128, 64:128] = W2
    W3stk = np.zeros((128, 2))
    W3stk[0:64, 0:1] = W3
    W3stk[64:128, 1:2] = W3
    return {
        "C1XY": pad48(mats["C1"]),          # [48,128]
        "Dvw": Dvw64,                        # [128,128]
        "XNDU": XNDU,                        # [48,48]
        "XND_W": XND_W64,                    # [128,16]
        "UW2": UW2,                          # [128,8]
        "UW3n": UW3n,                        # [128,8]
        "W1blk": W1blk, "W2blk": W2blk, "W3stk": W3stk,
    }


def pack_blob(mats):
    entries = ["C1XY", "Dvw", "XNDU", "XND_W", "UW2", "UW3n",
               "W1blk", "W2blk", "W3stk"]
    col = 0
    offsets = {}
    cols_total = sum(int(np.asarray(mats[e]).shape[1]) for e in entries)
    blob = np.zeros((128, cols_total), np.float32)
    for name in entries:
        M = np.asarray(mats[name])
        k, m = M.shape
        blob[0:k, col:col + m] = M.astype(np.float32)
        offsets[name] = (0, k, m, col)
        col += m
    return blob.astype(bf16), offsets


# ---------------------------------------------------------------------------
# numpy emulator of the exact kernel dataflow
# ---------------------------------------------------------------------------

def emulate(inp, t_steps=None):
    t_steps = t_steps or T
    r = lambda a: a.astype(bf16).astype(np.float32)
    mats = fold_matrices(inp)
    m2 = build_mats(inp)
    C1 = r(mats["C1"].astype(np.float32))
    XND_XY = r(pad48(mats["XND"]["XY"]).astype(np.float32))[
        np.r_[0:16, 32:48]]
    XND_W = r(m2["XND_W"].astype(np.float32))
    U_XY = r(pad48(mats["U"]["XY"]).astype(np.float32))[np.r_[0:16, 32:48]]
    UW2 = r(m2["UW2"].astype(np.float32))
    UW3n = r(m2["UW3n"].astype(np.float32))
    Dvw = r(mats["Dvw"].astype(np.float32))
    obs = np.asarray(inp["obs"], np.float32)
    ys = [r(obs[:, t, :]) for t in range(t_steps)]
    xs = {0: r(np.asarray(inp["x0"], np.float32))}
    means = np.zeros((B, t_steps, OUT), np.float32)
    xy0 = np.hstack([xs[0], ys[0]])
    w = np.zeros((B, NL), np.float32)
    for i in range(COLD):
        w = r(np.tanh(xy0 @ C1 + w @ Dvw))
    i0 = {0: w}
    i0[1] = r(np.tanh(xs[0] @ C1[:ST] + ys[1] @ C1[ST:] + i0[0] @ Dvw))
    i1, i2 = {}, {}
    for rr in range(t_steps // 2 + 1):
        t = 2 * rr
        for j in (0, 1):      # x chain
            tt = t + j
            if tt >= t_steps:
                break
            xy = np.hstack([xs[tt], ys[tt]])
            xs[tt + 1] = r(xs[tt] + xy @ XND_XY + i0[tt] @ XND_W)
        for j in (0, 1):      # refine 1
            tt = t + j
            if tt >= t_steps:
                break
            xy = np.hstack([xs[tt], ys[tt]])
            i1[tt] = r(np.tanh(xy @ C1 + i0[tt] @ Dvw))
        for j in (0, 1):      # refine 2 of previous round's steps + u
            tt = t - 2 + j
            if tt < 0 or tt >= t_steps:
                continue
            xy = np.hstack([xs[tt], ys[tt]])
            i2[tt] = r(np.tanh(xy @ C1 + i1[tt] @ Dvw))
            means[:, tt] = xy @ U_XY + i2[tt] @ UW2 + i1[tt] @ UW3n
        if t + 2 < t_steps:   # i0 wavefront columns (stale x, warm-diag)
            xcol = xs[max(0, t - 2)]
            for j in (0, 1):
                tp = t + 2 + j
                if tp >= t_steps:
                    break
                z = xcol @ C1[:ST] + ys[tp] @ C1[ST:] + i0[t + j] @ Dvw
                i0[tp] = r(np.tanh(z))
    W1m, W2m, W3m = (r(np.asarray(inp[k], np.float32))
                     for k in ("W1", "W2", "W3"))
    b1, b2, b3 = (np.asarray(inp[k], np.float32)
                  for k in ("b1", "b2", "b3"))
    of = r(obs.reshape(-1, IN))
    h = r(np.tanh(of @ W1m + b1))
    h = r(np.tanh(h @ W2m + b2))
    v = (h @ W3m + b3).reshape(B, T, 1)
    ls = np.broadcast_to(np.asarray(inp["log_stds"], np.float32),
                         means.shape)
    return np.concatenate([means, ls, v], -1)


# ---------------------------------------------------------------------------
# Bass program
# ---------------------------------------------------------------------------

def build_program(offsets, t_steps=None):
    import concourse.bacc as bacc
    import concourse.mybir as mybir
    from concourse import tile

    f32 = mybir.dt.float32
    bf = mybir.dt.bfloat16
    Tanh = mybir.ActivationFunctionType.Tanh
    Mul = mybir.AluOpType.mult
    Add = mybir.AluOpType.add

    nc = bacc.Bacc("TRN2", target_bir_lowering=False, debug=False,
                   num_devices=NCORES)

    cols_total = max(c + m for (_, _, m, c) in offsets.values())
    yin_d = nc.dram_tensor("yin", [4, IN, 2048], bf, kind="ExternalInput")
    omlp_d = nc.dram_tensor("omlp", [32, 4096], bf, kind="ExternalInput")
    x0_d = nc.dram_tensor("x0d", [ST, 512], bf, kind="ExternalInput")
    wb_d = nc.dram_tensor("wblob", [128, cols_total], bf,
                          kind="ExternalInput")
    bb_d = nc.dram_tensor("bblob", [128, 2], f32, kind="ExternalInput")
    means_d = nc.dram_tensor("means_o", [NR // 2, OUT, 1024], f32,
                             kind="ExternalOutput")
    value_d = nc.dram_tensor("value_o", [NG, 2, 512], f32,
                             kind="ExternalOutput")

    with tile.TileContext(nc) as tc:
        with (
            tc.tile_pool(name="const", bufs=1) as constp,
            tc.tile_pool(name="wi0", bufs=2) as wi0p,
            tc.tile_pool(name="w12", bufs=2) as w12p,
            tc.tile_pool(name="wcold", bufs=2) as wcoldp,
            tc.tile_pool(name="h", bufs=2) as hp,
            tc.tile_pool(name="mst", bufs=2) as mstp,
            tc.tile_pool(name="vst", bufs=2) as vstp,
            tc.tile_pool(name="pi0", bufs=2, space="PSUM") as pi0p,
            tc.tile_pool(name="p12", bufs=1, space="PSUM") as p12p,
            tc.tile_pool(name="sband2", bufs=1, space="PSUM") as sbp2,
            tc.tile_pool(name="sband", bufs=1, space="PSUM") as sbp,
            tc.tile_pool(name="mlpp", bufs=1, space="PSUM") as mlppp,
        ):
            # tanh table warmup overlapping the initial DMAs
            wu = constp.tile([1, 8], f32, tag="wu", name="wu")
            nc.vector.memset(wu[:], 0.0)
            wuo = constp.tile([1, 8], bf, tag="wuo", name="wuo")
            nc.scalar.activation(wuo[:], wu[:], Tanh)

            WB = constp.tile([128, cols_total], bf, tag="wb", name="WB")
            BB = constp.tile([128, 2], f32, tag="bb", name="BB")
            XYR = constp.tile([48, 512 * NR], bf, tag="xyr", name="XYR")
            OM = constp.tile([32, 4096], bf, tag="om", name="OM")

            def w_ap(name):
                row, k, m, c = offsets[name]
                return WB[row:row + k, c:c + m]

            solve_cols = offsets["W1blk"][3]
            nc.gpsimd.memset(XYR[0:32, :], 0.0)
            # cold-critical transfers first: the HWDGE queue runs in
            # emission order, so bulk y/MLP loads go after the cold chain
            nc.sync.dma_start(WB[:, 0:solve_cols], wb_d[:, 0:solve_cols])
            nc.sync.dma_start(XYR[0:ST, 0:512], x0_d[:])
            nc.sync.dma_start(XYR[32:48, 0:1024], yin_d[0][:, 0:1024])

            S = sbp.tile([128, 512], f32, tag="s", name="S")
            S2 = sbp2.tile([128, 512], f32, tag="s2", name="S2")
            MP = mlppp.tile([128, 512], f32, tag="mp", name="MP")
            def bulk_dmas():
                nc.sync.dma_start(XYR[32:48, 1024:2048],
                                  yin_d[0][:, 1024:2048])
                for g4 in range(1, 4):
                    nc.sync.dma_start(XYR[32:48, 2048 * g4:2048 * (g4 + 1)],
                                      yin_d[g4])
                nc.sync.dma_start(BB[:], bb_d[:])
                nc.sync.dma_start(WB[:, solve_cols:cols_total],
                                  wb_d[:, solve_cols:cols_total])
                nc.sync.dma_start(OM[:], omlp_d[:])

            # ---- cold start: COLD iterations for i0(0), then i0(1)
            wc = None
            Wi0 = wi0p.tile([128, 512], bf, tag="wi0", name="Wi0c")
            for i in range(COLD):
                pc = pi0p.tile([128, 512], f32, tag="pi0", name=f"pc{i}")
                last = i == COLD - 1
                nc.tensor.matmul(pc[:, 0:256], w_ap("C1XY"), XYR[:, 0:256],
                                 start=True, stop=(i == 0))
                if i > 0:
                    nc.tensor.matmul(pc[:, 0:256], w_ap("Dvw"), wc[:],
                                     start=False, stop=True)
                if last:
                    nc.scalar.activation(Wi0[:, 0:256], pc[:, 0:256], Tanh)
                else:
                    wc = wcoldp.tile([128, 256], bf, tag="wc", name=f"wc{i}")
                    nc.scalar.activation(wc[:], pc[:, 0:256], Tanh)
            pc = pi0p.tile([128, 512], f32, tag="pi0", name="pc_i01")
            nc.tensor.matmul(pc[:, 0:256], w_ap("C1XY"), XYR[:, 256:512],
                             start=True, stop=False)
            nc.tensor.matmul(pc[:, 0:256], w_ap("Dvw"), Wi0[:, 0:256],
                             start=False, stop=True)
            nc.scalar.activation(Wi0[:, 256:512], pc[:, 0:256], Tanh)
            bulk_dmas()

            def uslot(k):
                us = k % 4
                return (S if us < 2 else S2), 64 * (us % 2)

            W12 = None          # [i1pair | i2pair] of previous round
            h12 = h22 = None
            MSTt = None

            for rr in range(NR):
                t0 = 2 * rr
                q = rr % 2
                qn = (rr - 1) % 2
                s, sp_, sn = rr, max(rr - 1, 0), rr + 1
                cs = lambda k, a=0, b=512: slice(512 * k + a, 512 * k + b)
                last_round = rr == NR - 1
                Wi0p = Wi0            # i0(t0), i0(t0+1)
                W12p = W12            # i1pair(rr-1) | i2pair(rr-2)

                # stale-x dup into next slot's x-region for the i0 columns:
                # x(t0-2) (slot sp, written a full round ago) keeps the
                # x-chain off the T0->T0 critical cycle; round 0 uses x(0)
                if not last_round:
                    sx = s if rr == 0 else sp_
                    nc.vector.tensor_copy(XYR[0:16, cs(sn, 0, 256)],
                                          XYR[0:16, cs(sx, 0, 256)])
                    nc.vector.tensor_copy(XYR[0:16, cs(sn, 256, 512)],
                                          XYR[0:16, cs(sx, 0, 256)])
                    # T0 critical path: i0 psum + warm Dvw + tanh, emitted
                    # first so the engine queues prioritize the cycle
                    Pi0 = pi0p.tile([128, 512], f32, tag="pi0", name="Pi0")
                    nc.tensor.matmul(Pi0[:], w_ap("C1XY"), XYR[:, cs(sn)],
                                     start=True, stop=False)
                    nc.tensor.matmul(Pi0[:], w_ap("Dvw"), Wi0p[:],
                                     start=False, stop=True)
                    Wi0 = wi0p.tile([128, 512], bf, tag="wi0", name="Wi0")
                    nc.scalar.activation(Wi0[:], Pi0[:], Tanh)

                # deferred u-finish for round rr-2's steps (parity q): its
                # UW2 input i2pair(rr-2) was computed by LAST round's T12,
                # so everything here is ready when the PE queue reaches it
                # (an in-round UW2 would stall the queue behind T12)
                if rr >= 2:
                    SU2, ub2 = uslot(rr - 2)
                    nc.tensor.matmul(SU2[ub2:ub2 + 8, 0:512],
                                     w_ap("UW2"), W12p[:, 512:1024],
                                     start=False, stop=True)
                    half = (rr - 2) % 2
                    if half == 0:
                        MSTt = mstp.tile([OUT, 1024], f32, tag="mst",
                                         name="MST")
                    nc.vector.tensor_copy(
                        MSTt[:, 512 * half:512 * half + 512],
                        SU2[ub2:ub2 + 8, 0:512])
                    if half == 1:
                        nc.sync.dma_start(means_d[(rr - 2) // 2], MSTt[:])

                # xnd(t0) + u-xy(t0)  [uslot rows: u +0:8, xnd +32:48]
                SU, ub = uslot(rr)
                nc.tensor.matmul(SU[ub:ub + 48, 0:256],
                                 w_ap("XNDU"), XYR[:, cs(s, 0, 256)],
                                 start=True, stop=False)
                nc.tensor.matmul(SU[ub + 32:ub + 48, 0:256],
                                 w_ap("XND_W"), Wi0p[:, 0:256],
                                 start=False, stop=True,
                                 tile_position=(0, ub + 32))
                # x(t0+1) = x(t0) + xnd(t0)
                nc.vector.scalar_tensor_tensor(
                    XYR[0:16, cs(s, 256, 512)], XYR[0:16, cs(s, 0, 256)],
                    1.0, SU[ub + 32:ub + 48, 0:256], Mul, Add)

                # T12 psum: i1pair | i2pair.  The i1 C1 matmul is split so
                # only the x(t0+1)-dependent half sits behind upd#1; the
                # first start=True wipes bank A full-width so every later
                # MM accumulates with start=False.
                P12 = p12p.tile([128, 1024], f32, tag="p12", name="P12")
                nc.tensor.matmul(P12[:, 0:256], w_ap("C1XY"),
                                 XYR[:, cs(s, 0, 256)],
                                 start=True, stop=False)
                nc.tensor.matmul(P12[:, 0:512], w_ap("Dvw"), Wi0p[:],
                                 start=False, stop=False)
                if rr > 0:
                    nc.tensor.matmul(P12[:, 512:1024], w_ap("C1XY"),
                                     XYR[:, cs(sp_)], start=True, stop=False)
                    nc.tensor.matmul(P12[:, 512:1024], w_ap("Dvw"),
                                     W12p[:, 0:512], start=False, stop=True)
                    SUp, ubp = uslot(rr - 1)
                    # u(rr-1 steps): UW3n part (i1pair available now)
                    nc.tensor.matmul(SUp[ubp:ubp + 8, 0:512],
                                     w_ap("UW3n"), W12p[:, 0:512],
                                     start=False, stop=False)
                # x(t0+1)-dependent i1 half, emitted last among bank-A MMs
                nc.tensor.matmul(P12[:, 256:512], w_ap("C1XY"),
                                 XYR[:, cs(s, 256, 512)],
                                 start=False, stop=True)

                # xnd(t1) + u-xy(t1)
                # start=False: the t0 fused MM's start=True already cleared
                # these partitions bank-wide (PSUM start clears the full
                # bank row for written partitions)
                nc.tensor.matmul(SU[ub:ub + 48, 256:512],
                                 w_ap("XNDU"), XYR[:, cs(s, 256, 512)],
                                 start=False, stop=False)
                if not last_round:
                    nc.tensor.matmul(SU[ub + 32:ub + 48, 256:512],
                                     w_ap("XND_W"), Wi0p[:, 256:512],
                                     start=False, stop=True,
                                     tile_position=(0, ub + 32))
                    # x(t0+2) = x(t0+1) + xnd(t1); overwrites the stale-x
                    # dup AFTER this round's Pi0 C1 matmul read it (WAR)
                    nc.vector.scalar_tensor_tensor(
                        XYR[0:16, cs(sn, 0, 256)],
                        XYR[0:16, cs(s, 256, 512)], 1.0,
                        SU[ub + 32:ub + 48, 256:512], Mul, Add)

                # T12 tanh
                W12 = w12p.tile([128, 1024], bf, tag="w12", name="W12")
                if rr > 0:
                    nc.scalar.activation(W12[:], P12[:], Tanh)
                else:
                    nc.scalar.activation(W12[:, 0:512], P12[:, 0:512], Tanh)

                # value MLP
                g = rr // 2
                if rr % 2 == 0:
                    nc.tensor.matmul(MP[:], w_ap("W1blk"),
                                     OM[:, 512 * g:512 * g + 512],
                                     start=True, stop=True)
                    h12 = hp.tile([128, 512], bf, tag="h", name="h12")
                    nc.scalar.activation(h12[:], MP[:], Tanh,
                                         bias=BB[:, 0:1])
                else:
                    nc.tensor.matmul(MP[:], w_ap("W2blk"), h12[:],
                                     start=True, stop=True)
                    h22 = hp.tile([128, 512], bf, tag="h", name="h22")
                    nc.scalar.activation(h22[:], MP[:], Tanh,
                                         bias=BB[:, 1:2])
                    nc.tensor.matmul(MP[0:2, :], w_ap("W3stk"), h22[:],
                                     start=True, stop=True)
                    VST = vstp.tile([2, 512], f32, tag="vst", name="VST")
                    nc.vector.tensor_copy(VST[:], MP[0:2, :])
                    nc.sync.dma_start(value_d[g], VST[:])

            # ---- epilogue: u(14)-finish, then i2 + u for round 15's steps
            q15 = (NR - 1) % 2
            q14 = (NR - 2) % 2
            s15 = NR - 1
            SU14 = S2
            nc.tensor.matmul(SU14[0:8, 0:512],
                             w_ap("UW2"), W12[:, 512:1024],
                             start=False, stop=True)
            MSTt = mstp.tile([OUT, 1024], f32, tag="mst", name="MSTe")
            nc.vector.tensor_copy(MSTt[:, 0:512], SU14[0:8, 0:512])
            P12e = p12p.tile([128, 1024], f32, tag="p12", name="P12e")
            nc.tensor.matmul(P12e[:, 0:512], w_ap("C1XY"),
                             XYR[:, 512 * s15:512 * s15 + 512],
                             start=True, stop=False)
            nc.tensor.matmul(P12e[:, 0:512], w_ap("Dvw"), W12[:, 0:512],
                             start=False, stop=True)
            nc.tensor.matmul(S2[64:72, 0:512],
                             w_ap("UW3n"), W12[:, 0:512],
                             start=False, stop=False)
            Wi2e = wi0p.tile([128, 512], bf, tag="wi0", name="Wi2e")
            nc.scalar.activation(Wi2e[:], P12e[:, 0:512], Tanh)
            nc.tensor.matmul(S2[64:72, 0:512],
                             w_ap("UW2"), Wi2e[:], start=False, stop=True)
            nc.vector.tensor_copy(MSTt[:, 512:1024], S2[64:72, 0:512])
            nc.sync.dma_start(means_d[NR // 2 - 1], MSTt[:])
    nc.compile()
    return nc


# ---------------------------------------------------------------------------
# host glue
# ---------------------------------------------------------------------------

def _prep_inputs(inputs):
    obs = np.asarray(inputs["obs"], np.float32)
    x0 = np.asarray(inputs["x0"], np.float32)
    mats = build_mats(inputs)
    blob, offsets = pack_blob(mats)
    bb = np.zeros((128, 2), np.float32)
    b1 = np.asarray(inputs["b1"], np.float32)
    b2 = np.asarray(inputs["b2"], np.float32)
    bb[0:H, 0] = bb[H:2 * H, 0] = b1
    bb[0:H, 1] = bb[H:2 * H, 1] = b2

    in_maps = []
    for m in range(NCORES):
        osh = obs[m * BL:(m + 1) * BL]            # [BL, T, IN]
        obs_t = np.ascontiguousarray(osh.transpose(1, 2, 0))  # [T, IN, BL]
        yin = obs_t.reshape(NR, 2, IN, BL).transpose(0, 2, 1, 3) \
            .reshape(NR, IN, 512).astype(bf16)
        yin4 = np.ascontiguousarray(
            yin.reshape(4, 4, IN, 512).transpose(0, 2, 1, 3)
            .reshape(4, IN, 2048))
        omlp = np.zeros((32, 4096), np.float32)
        for g in range(NG):
            omlp[0:16, 512 * g:512 * (g + 1)] = yin[2 * g].astype(np.float32)
            omlp[16:32, 512 * g:512 * (g + 1)] = \
                yin[2 * g + 1].astype(np.float32)
        x0t = np.ascontiguousarray(x0[m * BL:(m + 1) * BL].T)  # [16,256]
        x0d = np.concatenate([x0t, x0t], axis=1).astype(bf16)  # [16,512]
        in_maps.append({
            "yin": yin4,
            "omlp": omlp.astype(bf16),
            "x0d": x0d, "wblob": blob, "bblob": bb,
        })
    return in_maps, offsets


def assemble(inputs, results):
    means = np.zeros((B, T, OUT), np.float32)
    value = np.zeros((B, T, 1), np.float32)
    b3 = float(np.asarray(inputs["b3"], np.float32).ravel()[0])
    for m, r in enumerate(results):
        md = r["means_o"]            # [8, OUT, 1024]
        md = md.reshape(NR // 2, OUT, 2, 2, BL)
        # axes (k, o, h, j, b) -> t = 4k + 2h + j
        means[m * BL:(m + 1) * BL] = md.transpose(4, 0, 2, 3, 1) \
            .reshape(BL, T, OUT)
        vd = r["value_o"].reshape(NG, 2, 2, BL)   # (g, blk, dt, b)
        value[m * BL:(m + 1) * BL, :, 0] = \
            vd.transpose(3, 0, 1, 2).reshape(BL, T) + b3
    ls = np.broadcast_to(
        np.asarray(inputs["log_stds"], np.float32), means.shape)
    return np.concatenate([means, ls, value], -1)


def run(inputs, t_steps=None, trace=False):
    from concourse.bass_utils import run_bass_kernel_spmd
    in_maps, offsets = _prep_inputs(inputs)
    nc = build_program(offsets)
    res = run_bass_kernel_spmd(nc, in_maps, list(range(NCORES)),
                               trace=trace)
    return res


def kernel(**inputs):
    res = run(inputs)
    return assemble(inputs, res.results)


if __name__ == "__main__":
    pass


# revision 3
# speedup vs baseline: 1.0273x; 1.0273x over previous
"""Trainium2 Bass kernel v2 for the DissipativeRINN problem.

Structure (pure data parallel over batch, 8 cores x 256 batch each,
transposed on-chip layout [feature, batch]):
  - 2-step ROUNDS: round rr handles steps t0=2rr, t0+1.  Per round,
    THREE solve tanh columns per step (i0 wavefront + 2 refinements):
      T0  = tanh [128,512]  : i0(t0+2), i0(t0+3)   (next round's warm
            iterates; stale-x = x(t0), warm-diag from i0(t0), i0(t0+1))
      T12 = tanh [128,1024] : [i1(t0), i1(t0+1) | i2(t0-2), i2(t0-1)]
    The controller output u(t) uses the Anderson-corrected iterate
      w* ~= i2 + (i2 - i1) @ Mc,  Mc = mu Dvw (I - mu Dvw)^-1,  mu=0.7
    folded into two stationary matrices UW2/UW3n (one extra matmul, no
    extra tanh).  Emulator rel err vs reference: 6.5e-3 (gate 2e-2).
  - x-chain (RK4 state update, matrices folded host-side as in v1) rides
    on i0 only, off the tanh critical cycle; x carried bf16 in a 3-slot
    XY ring [48, 512/slot] (x rows 0:16, y rows 32:48, y DMA'd direct).
  - Small-matmul fusions: [U_XY | XND_XY] share one stationary (output
    partition-stacked 0:8 u / 32:48 xnd in one PSUM bank with parity
    slots at base 0/64); value MLP uses block-diagonal W1/W2 (2 token
    blocks per matmul) and partition-stacked W3 ([128,2]).
  - PSUM budget exactly 8 banks: 2x[128,512] i0, 2x[128,1024] T12,
    1 shared xnd/u bank, 1 MLP bank.
"""

import os

import numpy as np
import ml_dtypes

bf16 = ml_dtypes.bfloat16

DT = 0.01
B, T, IN, ST, NL, OUT, H = 2048, 32, 16, 16, 128, 8, 64
NCORES = 8
BL = B // NCORES      # 256
NR = T // 2           # 16 rounds
NG = T // 4           # 8 MLP groups
COLD = int(os.environ.get("K_COLD", "3"))
MU = float(os.environ.get("K_MU", "0.7"))
T_STEPS = T           # test.py compatibility
BB_B3 = 0.0


# ---------------------------------------------------------------------------
# host-side folding (identical math to v1) + v2 matrices
# ---------------------------------------------------------------------------

def _dadd(*ds):
    out = {}
    for d in ds:
        for k, v in d.items():
            out[k] = out.get(k, 0) + v
    return out


def _dmul(d, M):
    return {k: v @ M for k, v in d.items()}


def _dscale(d, s):
    return {k: s * v for k, v in d.items()}


def fold_matrices(inp):
    f64 = lambda k: np.asarray(inp[k], np.float64)
    A_T, Bw_T, By_T = f64("A_T"), f64("Bw_T"), f64("By_T")
    Cv_T, Dvw_T, Dvy_T = f64("Cv_T"), f64("Dvw_T"), f64("Dvy_T")
    Cu_T, Duw_T, Duy_T = f64("Cu_T"), f64("Duw_T"), f64("Duy_T")
    I16 = np.eye(16)
    Z16 = np.zeros((16, 16))
    X = {"XY": np.vstack([I16, Z16])}
    Y = {"XY": np.vstack([Z16, I16])}

    def K_of(Xd, s):
        return _dadd(_dmul(Xd, A_T), _dmul(Y, By_T), {f"W{s}": Bw_T})

    K1 = K_of(X, 1)
    X2 = _dadd(X, _dscale(K1, DT / 2))
    K2 = K_of(X2, 2)
    X3 = _dadd(X, _dscale(K2, DT / 2))
    K3 = K_of(X3, 3)
    X4 = _dadd(X, _dscale(K3, DT))
    K4 = K_of(X4, 4)
    XND = _dscale(_dadd(K1, _dscale(K2, 2), _dscale(K3, 2), K4), DT / 6)
    C1 = _dadd(_dmul(X, Cv_T), _dmul(Y, Dvy_T))["XY"]
    U = _dadd({"XY": np.vstack([Cu_T, Duy_T])}, {"W1": Duw_T})
    return {"C1": C1, "XND": XND, "U": U, "Dvw": Dvw_T}


def pad48(M):
    M = np.asarray(M)
    out = np.zeros((48, M.shape[1]))
    out[0:ST] = M[0:ST]
    out[32:48] = M[ST:2 * ST]
    return out


def build_mats(inp):
    mats = fold_matrices(inp)
    Dvw64 = mats["Dvw"]
    XND_W64 = sum(mats["XND"][f"W{s}"] for s in range(1, 5))
    Mc = (MU * Dvw64) @ np.linalg.inv(np.eye(NL) - MU * Dvw64)
    UW2 = (np.eye(NL) + Mc) @ mats["U"]["W1"]
    UW3n = -(Mc @ mats["U"]["W1"])
    XNDU = np.zeros((48, 48))
    XNDU[:, 0:8] = pad48(mats["U"]["XY"])
    XNDU[:, 32:48] = pad48(mats["XND"]["XY"])
    W1 = np.asarray(inp["W1"], np.float64)
    W2 = np.asarray(inp["W2"], np.float64)
    W3 = np.asarray(inp["W3"], np.float64)
    W1blk = np.zeros((32, 128))
    W1blk[0:16, 0:64] = W1
    W1blk[16:32, 64:128] = W1
    W2blk = np.zeros((128, 128))
    W2blk[0:64, 0:64] = W2
    W2blk[64:128, 64:128] = W2
    W3stk = np.zeros((128, 2))
    W3stk[0:64, 0:1] = W3
    W3stk[64:128, 1:2] = W3
    return {
        "C1XY": pad48(mats["C1"]),          # [48,128]
        "Dvw": Dvw64,                        # [128,128]
        "XNDU": XNDU,                        # [48,48]
        "XND_W": XND_W64,                    # [128,16]
        "UW2": UW2,                          # [128,8]
        "UW3n": UW3n,                        # [128,8]
        "W1blk": W1blk, "W2blk": W2blk, "W3stk": W3stk,
    }


def pack_blob(mats):
    entries = ["C1XY", "Dvw", "XNDU", "XND_W", "UW2", "UW3n",
               "W1blk", "W2blk", "W3stk"]
    col = 0
    offsets = {}
    cols_total = sum(int(np.asarray(mats[e]).shape[1]) for e in entries)
    blob = np.zeros((128, cols_total), np.float32)
    for name in entries:
        M = np.asarray(mats[name])
        k, m = M.shape
        blob[0:k, col:col + m] = M.astype(np.float32)
        offsets[name] = (0, k, m, col)
        col += m
    return blob.astype(bf16), offsets


# ---------------------------------------------------------------------------
# numpy emulator of the exact kernel dataflow
# ---------------------------------------------------------------------------

def emulate(inp, t_steps=None):
    t_steps = t_steps or T
    r = lambda a: a.astype(bf16).astype(np.float32)
    mats = fold_matrices(inp)
    m2 = build_mats(inp)
    C1 = r(mats["C1"].astype(np.float32))
    XND_XY = r(pad48(mats["XND"]["XY"]).astype(np.float32))[
        np.r_[0:16, 32:48]]
    XND_W = r(m2["XND_W"].astype(np.float32))
    U_XY = r(pad48(mats["U"]["XY"]).astype(np.float32))[np.r_[0:16, 32:48]]
    UW2 = r(m2["UW2"].astype(np.float32))
    UW3n = r(m2["UW3n"].astype(np.float32))
    Dvw = r(mats["Dvw"].astype(np.float32))
    obs = np.asarray(inp["obs"], np.float32)
    ys = [r(obs[:, t, :]) for t in range(t_steps)]
    xs = {0: r(np.asarray(inp["x0"], np.float32))}
    means = np.zeros((B, t_steps, OUT), np.float32)
    xy0 = np.hstack([xs[0], ys[0]])
    w = np.zeros((B, NL), np.float32)
    for i in range(COLD):
        w = r(np.tanh(xy0 @ C1 + w @ Dvw))
    i0 = {0: w}
    i0[1] = r(np.tanh(xs[0] @ C1[:ST] + ys[1] @ C1[ST:] + i0[0] @ Dvw))
    i1, i2 = {}, {}
    for rr in range(t_steps // 2 + 1):
        t = 2 * rr
        for j in (0, 1):      # x chain
            tt = t + j
            if tt >= t_steps:
                break
            xy = np.hstack([xs[tt], ys[tt]])
            xs[tt + 1] = r(xs[tt] + xy @ XND_XY + i0[tt] @ XND_W)
        for j in (0, 1):      # refine 1
            tt = t + j
            if tt >= t_steps:
                break
            xy = np.hstack([xs[tt], ys[tt]])
            i1[tt] = r(np.tanh(xy @ C1 + i0[tt] @ Dvw))
        for j in (0, 1):      # refine 2 of previous round's steps + u
            tt = t - 2 + j
            if tt < 0 or tt >= t_steps:
                continue
            xy = np.hstack([xs[tt], ys[tt]])
            i2[tt] = r(np.tanh(xy @ C1 + i1[tt] @ Dvw))
            means[:, tt] = xy @ U_XY + i2[tt] @ UW2 + i1[tt] @ UW3n
        if t + 2 < t_steps:   # i0 wavefront columns (stale x, warm-diag)
            xcol = xs[max(0, t - 2)]
            for j in (0, 1):
                tp = t + 2 + j
                if tp >= t_steps:
                    break
                z = xcol @ C1[:ST] + ys[tp] @ C1[ST:] + i0[t + j] @ Dvw
                i0[tp] = r(np.tanh(z))
    W1m, W2m, W3m = (r(np.asarray(inp[k], np.float32))
                     for k in ("W1", "W2", "W3"))
    b1, b2, b3 = (np.asarray(inp[k], np.float32)
                  for k in ("b1", "b2", "b3"))
    of = r(obs.reshape(-1, IN))
    h = r(np.tanh(of @ W1m + b1))
    h = r(np.tanh(h @ W2m + b2))
    v = (h @ W3m + b3).reshape(B, T, 1)
    ls = np.broadcast_to(np.asarray(inp["log_stds"], np.float32),
                         means.shape)
    return np.concatenate([means, ls, v], -1)


# ---------------------------------------------------------------------------
# Bass program
# ---------------------------------------------------------------------------

def build_program(offsets, t_steps=None):
    import concourse.bacc as bacc
    import concourse.mybir as mybir
    from concourse import tile

    f32 = mybir.dt.float32
    bf = mybir.dt.bfloat16
    Tanh = mybir.ActivationFunctionType.Tanh
    Mul = mybir.AluOpType.mult
    Add = mybir.AluOpType.add

    nc = bacc.Bacc("TRN2", target_bir_lowering=False, debug=False,
                   num_devices=NCORES)

    cols_total = max(c + m for (_, _, m, c) in offsets.values())
    yin_d = nc.dram_tensor("yin", [4, IN, 2048], bf, kind="ExternalInput")
    omlp_d = nc.dram_tensor("omlp", [32, 4096], bf, kind="ExternalInput")
    init_d = nc.dram_tensor("initd", [48, 1024], bf, kind="ExternalInput")
    zp_d = nc.dram_tensor("zpad", [16, 512 * NR], bf, kind="ExternalInput")
    wb_d = nc.dram_tensor("wblob", [128, cols_total], bf,
                          kind="ExternalInput")
    bb_d = nc.dram_tensor("bblob", [128, 2], f32, kind="ExternalInput")
    means_d = nc.dram_tensor("means_o", [NR // 2, OUT, 1024], f32,
                             kind="ExternalOutput")
    value_d = nc.dram_tensor("value_o", [NG, 2, 512], f32,
                             kind="ExternalOutput")

    with tile.TileContext(nc) as tc:
        with (
            tc.tile_pool(name="const", bufs=1) as constp,
            tc.tile_pool(name="wi0", bufs=2) as wi0p,
            tc.tile_pool(name="w12", bufs=2) as w12p,
            tc.tile_pool(name="wcold", bufs=2) as wcoldp,
            tc.tile_pool(name="h", bufs=2) as hp,
            tc.tile_pool(name="mst", bufs=2) as mstp,
            tc.tile_pool(name="vst", bufs=2) as vstp,
            tc.tile_pool(name="pi0", bufs=2, space="PSUM") as pi0p,
            tc.tile_pool(name="p12", bufs=1, space="PSUM") as p12p,
            tc.tile_pool(name="sband2", bufs=1, space="PSUM") as sbp2,
            tc.tile_pool(name="sband", bufs=1, space="PSUM") as sbp,
            tc.tile_pool(name="mlpp", bufs=1, space="PSUM") as mlppp,
        ):
            # tanh table warmup overlapping the initial DMAs
            wu = constp.tile([1, 8], f32, tag="wu", name="wu")
            nc.vector.memset(wu[:], 0.0)
            wuo = constp.tile([1, 8], bf, tag="wuo", name="wuo")
            nc.scalar.activation(wuo[:], wu[:], Tanh)

            WB = constp.tile([128, cols_total], bf, tag="wb", name="WB")
            BB = constp.tile([128, 2], f32, tag="bb", name="BB")
            XYR = constp.tile([48, 512 * NR], bf, tag="xyr", name="XYR")
            OM = constp.tile([32, 4096], bf, tag="om", name="OM")

            def w_ap(name):
                row, k, m, c = offsets[name]
                return WB[row:row + k, c:c + m]

            solve_cols = offsets["W1blk"][3]
            # pad band rows 16:32 zeroed via DMA (a gpsimd memset has a
            # 32-partition alignment rule, and memsetting rows 0:32 stalls
            # the x0 DMA ~7us behind the Pool engine).  x-rows need no
            # zeroing: every slot's x-region is written before it is read.
            # Cold-critical transfers first: the HWDGE queue runs in
            # emission order, so bulk y/MLP loads go after the cold chain.
            nc.sync.dma_start(WB[:, 0:solve_cols], wb_d[:, 0:solve_cols])
            nc.sync.dma_start(XYR[0:48, 0:1024], init_d[:])

            S = sbp.tile([128, 512], f32, tag="s", name="S")
            S2 = sbp2.tile([128, 512], f32, tag="s2", name="S2")
            MP = mlppp.tile([128, 512], f32, tag="mp", name="MP")
            def bulk_dmas():
                nc.sync.dma_start(XYR[16:32, 1024:512 * NR],
                                  zp_d[:, 1024:512 * NR])
                nc.sync.dma_start(XYR[32:48, 1024:2048],
                                  yin_d[0][:, 1024:2048])
                for g4 in range(1, 4):
                    nc.sync.dma_start(XYR[32:48, 2048 * g4:2048 * (g4 + 1)],
                                      yin_d[g4])
                nc.sync.dma_start(BB[:], bb_d[:])
                nc.sync.dma_start(WB[:, solve_cols:cols_total],
                                  wb_d[:, solve_cols:cols_total])
                nc.sync.dma_start(OM[:], omlp_d[:])

            # ---- cold start: COLD iterations for i0(0), then i0(1)
            wc = None
            Wi0 = wi0p.tile([128, 512], bf, tag="wi0", name="Wi0c")
            for i in range(COLD):
                pc = pi0p.tile([128, 512], f32, tag="pi0", name=f"pc{i}")
                last = i == COLD - 1
                nc.tensor.matmul(pc[:, 0:256], w_ap("C1XY"), XYR[:, 0:256],
                                 start=True, stop=(i == 0))
                if i > 0:
                    nc.tensor.matmul(pc[:, 0:256], w_ap("Dvw"), wc[:],
                                     start=False, stop=True)
                if last:
                    nc.scalar.activation(Wi0[:, 0:256], pc[:, 0:256], Tanh)
                else:
                    wc = wcoldp.tile([128, 256], bf, tag="wc", name=f"wc{i}")
                    nc.scalar.activation(wc[:], pc[:, 0:256], Tanh)
            pc = pi0p.tile([128, 512], f32, tag="pi0", name="pc_i01")
            nc.tensor.matmul(pc[:, 0:256], w_ap("C1XY"), XYR[:, 256:512],
                             start=True, stop=False)
            nc.tensor.matmul(pc[:, 0:256], w_ap("Dvw"), Wi0[:, 0:256],
                             start=False, stop=True)
            nc.scalar.activation(Wi0[:, 256:512], pc[:, 0:256], Tanh)
            bulk_dmas()

            def uslot(k):
                us = k % 4
                return (S if us < 2 else S2), 64 * (us % 2)

            W12 = None          # [i1pair | i2pair] of previous round
            h12 = h22 = None
            MSTt = None

            for rr in range(NR):
                t0 = 2 * rr
                q = rr % 2
                qn = (rr - 1) % 2
                s, sp_, sn = rr, max(rr - 1, 0), rr + 1
                cs = lambda k, a=0, b=512: slice(512 * k + a, 512 * k + b)
                last_round = rr == NR - 1
                Wi0p = Wi0            # i0(t0), i0(t0+1)
                W12p = W12            # i1pair(rr-1) | i2pair(rr-2)

                # stale-x dup into next slot's x-region for the i0 columns:
                # x(t0-2) (slot sp, written a full round ago) keeps the
                # x-chain off the T0->T0 critical cycle; round 0 uses x(0)
                if not last_round:
                    sx = s if rr == 0 else sp_
                    nc.vector.tensor_copy(XYR[0:16, cs(sn, 0, 256)],
                                          XYR[0:16, cs(sx, 0, 256)])
                    nc.vector.tensor_copy(XYR[0:16, cs(sn, 256, 512)],
                                          XYR[0:16, cs(sx, 0, 256)])
                    # T0 critical path: i0 psum + warm Dvw + tanh, emitted
                    # first so the engine queues prioritize the cycle
                    Pi0 = pi0p.tile([128, 512], f32, tag="pi0", name="Pi0")
                    nc.tensor.matmul(Pi0[:], w_ap("C1XY"), XYR[:, cs(sn)],
                                     start=True, stop=False)
                    nc.tensor.matmul(Pi0[:], w_ap("Dvw"), Wi0p[:],
                                     start=False, stop=True)
                    Wi0 = wi0p.tile([128, 512], bf, tag="wi0", name="Wi0")
                    nc.scalar.activation(Wi0[:], Pi0[:], Tanh)

                # deferred u-finish for round rr-2's steps (parity q): its
                # UW2 input i2pair(rr-2) was computed by LAST round's T12,
                # so everything here is ready when the PE queue reaches it
                # (an in-round UW2 would stall the queue behind T12)
                if rr >= 2:
                    SU2, ub2 = uslot(rr - 2)
                    nc.tensor.matmul(SU2[ub2:ub2 + 8, 0:512],
                                     w_ap("UW2"), W12p[:, 512:1024],
                                     start=False, stop=True)
                    half = (rr - 2) % 2
                    if half == 0:
                        MSTt = mstp.tile([OUT, 1024], f32, tag="mst",
                                         name="MST")
                    nc.vector.tensor_copy(
                        MSTt[:, 512 * half:512 * half + 512],
                        SU2[ub2:ub2 + 8, 0:512])
                    if half == 1:
                        nc.sync.dma_start(means_d[(rr - 2) // 2], MSTt[:])

                # xnd(t0) + u-xy(t0)  [uslot rows: u +0:8, xnd +32:48]
                SU, ub = uslot(rr)
                nc.tensor.matmul(SU[ub:ub + 48, 0:256],
                                 w_ap("XNDU"), XYR[:, cs(s, 0, 256)],
                                 start=True, stop=False)
                nc.tensor.matmul(SU[ub + 32:ub + 48, 0:256],
                                 w_ap("XND_W"), Wi0p[:, 0:256],
                                 start=False, stop=True,
                                 tile_position=(0, ub + 32))
                # x(t0+1) = x(t0) + xnd(t0)
                nc.vector.scalar_tensor_tensor(
                    XYR[0:16, cs(s, 256, 512)], XYR[0:16, cs(s, 0, 256)],
                    1.0, SU[ub + 32:ub + 48, 0:256], Mul, Add)

                # T12 psum: i1pair | i2pair.  The i1 C1 matmul is split so
                # only the x(t0+1)-dependent half sits behind upd#1; the
                # first start=True wipes bank A full-width so every later
                # MM accumulates with start=False.
                P12 = p12p.tile([128, 1024], f32, tag="p12", name="P12")
                nc.tensor.matmul(P12[:, 0:256], w_ap("C1XY"),
                                 XYR[:, cs(s, 0, 256)],
                                 start=True, stop=False)
                nc.tensor.matmul(P12[:, 0:512], w_ap("Dvw"), Wi0p[:],
                                 start=False, stop=False)
                if rr > 0:
                    nc.tensor.matmul(P12[:, 512:1024], w_ap("C1XY"),
                                     XYR[:, cs(sp_)], start=True, stop=False)
                    nc.tensor.matmul(P12[:, 512:1024], w_ap("Dvw"),
                                     W12p[:, 0:512], start=False, stop=True)
                    SUp, ubp = uslot(rr - 1)
                    # u(rr-1 steps): UW3n part (i1pair available now)
                    nc.tensor.matmul(SUp[ubp:ubp + 8, 0:512],
                                     w_ap("UW3n"), W12p[:, 0:512],
                                     start=False, stop=False)
                # x(t0+1)-dependent i1 half, emitted last among bank-A MMs
                nc.tensor.matmul(P12[:, 256:512], w_ap("C1XY"),
                                 XYR[:, cs(s, 256, 512)],
                                 start=False, stop=True)

                # xnd(t1) + u-xy(t1)
                # start=False: the t0 fused MM's start=True already cleared
                # these partitions bank-wide (PSUM start clears the full
                # bank row for written partitions)
                nc.tensor.matmul(SU[ub:ub + 48, 256:512],
                                 w_ap("XNDU"), XYR[:, cs(s, 256, 512)],
                                 start=False, stop=False)
                if not last_round:
                    nc.tensor.matmul(SU[ub + 32:ub + 48, 256:512],
                                     w_ap("XND_W"), Wi0p[:, 256:512],
                                     start=False, stop=True,
                                     tile_position=(0, ub + 32))
                    # x(t0+2) = x(t0+1) + xnd(t1); overwrites the stale-x
                    # dup AFTER this round's Pi0 C1 matmul read it (WAR)
                    nc.vector.scalar_tensor_tensor(
                        XYR[0:16, cs(sn, 0, 256)],
                        XYR[0:16, cs(s, 256, 512)], 1.0,
                        SU[ub + 32:ub + 48, 256:512], Mul, Add)

                # T12 tanh
                W12 = w12p.tile([128, 1024], bf, tag="w12", name="W12")
                if rr > 0:
                    nc.scalar.activation(W12[:], P12[:], Tanh)
                else:
                    nc.scalar.activation(W12[:, 0:512], P12[:, 0:512], Tanh)

                # value MLP
                g = rr // 2
                if rr % 2 == 0:
                    nc.tensor.matmul(MP[:], w_ap("W1blk"),
                                     OM[:, 512 * g:512 * g + 512],
                                     start=True, stop=True)
                    h12 = hp.tile([128, 512], bf, tag="h", name="h12")
                    nc.scalar.activation(h12[:], MP[:], Tanh,
                                         bias=BB[:, 0:1])
                else:
                    nc.tensor.matmul(MP[:], w_ap("W2blk"), h12[:],
                                     start=True, stop=True)
                    h22 = hp.tile([128, 512], bf, tag="h", name="h22")
                    nc.scalar.activation(h22[:], MP[:], Tanh,
                                         bias=BB[:, 1:2])
                    nc.tensor.matmul(MP[0:2, :], w_ap("W3stk"), h22[:],
                                     start=True, stop=True)
                    VST = vstp.tile([2, 512], f32, tag="vst", name="VST")
                    nc.vector.tensor_copy(VST[:], MP[0:2, :])
                    nc.sync.dma_start(value_d[g], VST[:])

            # ---- epilogue: u(14)-finish, then i2 + u for round 15's steps
            q15 = (NR - 1) % 2
            q14 = (NR - 2) % 2
            s15 = NR - 1
            SU14 = S2
            nc.tensor.matmul(SU14[0:8, 0:512],
                             w_ap("UW2"), W12[:, 512:1024],
                             start=False, stop=True)
            MSTt = mstp.tile([OUT, 1024], f32, tag="mst", name="MSTe")
            nc.vector.tensor_copy(MSTt[:, 0:512], SU14[0:8, 0:512])
            P12e = p12p.tile([128, 1024], f32, tag="p12", name="P12e")
            nc.tensor.matmul(P12e[:, 0:512], w_ap("C1XY"),
                             XYR[:, 512 * s15:512 * s15 + 512],
                             start=True, stop=False)
            nc.tensor.matmul(P12e[:, 0:512], w_ap("Dvw"), W12[:, 0:512],
                             start=False, stop=True)
            nc.tensor.matmul(S2[64:72, 0:512],
                             w_ap("UW3n"), W12[:, 0:512],
                             start=False, stop=False)
            Wi2e = wi0p.tile([128, 512], bf, tag="wi0", name="Wi2e")
            nc.scalar.activation(Wi2e[:], P12e[:, 0:512], Tanh)
            nc.tensor.matmul(S2[64:72, 0:512],
                             w_ap("UW2"), Wi2e[:], start=False, stop=True)
            nc.vector.tensor_copy(MSTt[:, 512:1024], S2[64:72, 0:512])
            nc.sync.dma_start(means_d[NR // 2 - 1], MSTt[:])
    nc.compile()
    return nc


# ---------------------------------------------------------------------------
# host glue
# ---------------------------------------------------------------------------

def _prep_inputs(inputs):
    obs = np.asarray(inputs["obs"], np.float32)
    x0 = np.asarray(inputs["x0"], np.float32)
    mats = build_mats(inputs)
    blob, offsets = pack_blob(mats)
    bb = np.zeros((128, 2), np.float32)
    b1 = np.asarray(inputs["b1"], np.float32)
    b2 = np.asarray(inputs["b2"], np.float32)
    bb[0:H, 0] = bb[H:2 * H, 0] = b1
    bb[0:H, 1] = bb[H:2 * H, 1] = b2

    in_maps = []
    for m in range(NCORES):
        osh = obs[m * BL:(m + 1) * BL]            # [BL, T, IN]
        obs_t = np.ascontiguousarray(osh.transpose(1, 2, 0))  # [T, IN, BL]
        yin = obs_t.reshape(NR, 2, IN, BL).transpose(0, 2, 1, 3) \
            .reshape(NR, IN, 512).astype(bf16)
        yin4 = np.ascontiguousarray(
            yin.reshape(4, 4, IN, 512).transpose(0, 2, 1, 3)
            .reshape(4, IN, 2048))
        omlp = np.zeros((32, 4096), np.float32)
        for g in range(NG):
            omlp[0:16, 512 * g:512 * (g + 1)] = yin[2 * g].astype(np.float32)
            omlp[16:32, 512 * g:512 * (g + 1)] = \
                yin[2 * g + 1].astype(np.float32)
        x0t = np.ascontiguousarray(x0[m * BL:(m + 1) * BL].T)  # [16,256]
        init = np.zeros((48, 1024), np.float32)
        init[0:16, 0:256] = x0t
        init[0:16, 256:512] = x0t
        init[32:48, :] = yin4[0][:, 0:1024].astype(np.float32)
        in_maps.append({
            "yin": yin4,
            "zpad": np.zeros((16, 512 * NR), bf16),
            "omlp": omlp.astype(bf16),
            "initd": init.astype(bf16), "wblob": blob, "bblob": bb,
        })
    return in_maps, offsets


def assemble(inputs, results):
    means = np.zeros((B, T, OUT), np.float32)
    value = np.zeros((B, T, 1), np.float32)
    b3 = float(np.asarray(inputs["b3"], np.float32).ravel()[0])
    for m, r in enumerate(results):
        md = r["means_o"]            # [8, OUT, 1024]
        md = md.reshape(NR // 2, OUT, 2, 2, BL)
        # axes (k, o, h, j, b) -> t = 4k + 2h + j
        means[m * BL:(m + 1) * BL] = md.transpose(4, 0, 2, 3, 1) \
            .reshape(BL, T, OUT)
        vd = r["value_o"].reshape(NG, 2, 2, BL)   # (g, blk, dt, b)
        value[m * BL:(m + 1) * BL, :, 0] = \
            vd.transpose(3, 0, 1, 2).reshape(BL, T) + b3
    ls = np.broadcast_to(
        np.asarray(inputs["log_stds"], np.float32), means.shape)
    return np.concatenate([means, ls, value], -1)


def run(inputs, t_steps=None, trace=False):
    from concourse.bass_utils import run_bass_kernel_spmd
    in_maps, offsets = _prep_inputs(inputs)
    nc = build_program(offsets)
    res = run_bass_kernel_spmd(nc, in_maps, list(range(NCORES)),
                               trace=trace)
    return res


def kernel(**inputs):
    res = run(inputs)
    return assemble(inputs, res.results)


if __name__ == "__main__":
    pass


# revision 4
# speedup vs baseline: 1.0279x; 1.0005x over previous
"""Trainium2 Bass kernel v2 for the DissipativeRINN problem.

Structure (pure data parallel over batch, 8 cores x 256 batch each,
transposed on-chip layout [feature, batch]):
  - 2-step ROUNDS: round rr handles steps t0=2rr, t0+1.  Per round,
    THREE solve tanh columns per step (i0 wavefront + 2 refinements):
      T0  = tanh [128,512]  : i0(t0+2), i0(t0+3)   (next round's warm
            iterates; stale-x = x(t0), warm-diag from i0(t0), i0(t0+1))
      T12 = tanh [128,1024] : [i1(t0), i1(t0+1) | i2(t0-2), i2(t0-1)]
    The controller output u(t) uses the Anderson-corrected iterate
      w* ~= i2 + (i2 - i1) @ Mc,  Mc = mu Dvw (I - mu Dvw)^-1,  mu=0.7
    folded into two stationary matrices UW2/UW3n (one extra matmul, no
    extra tanh).  Emulator rel err vs reference: 6.5e-3 (gate 2e-2).
  - x-chain (RK4 state update, matrices folded host-side as in v1) rides
    on i0 only, off the tanh critical cycle; x carried bf16 in a 3-slot
    XY ring [48, 512/slot] (x rows 0:16, y rows 32:48, y DMA'd direct).
  - Small-matmul fusions: [U_XY | XND_XY] share one stationary (output
    partition-stacked 0:8 u / 32:48 xnd in one PSUM bank with parity
    slots at base 0/64); value MLP uses block-diagonal W1/W2 (2 token
    blocks per matmul) and partition-stacked W3 ([128,2]).
  - PSUM budget exactly 8 banks: 2x[128,512] i0, 2x[128,1024] T12,
    1 shared xnd/u bank, 1 MLP bank.
"""

import os

import numpy as np
import ml_dtypes

bf16 = ml_dtypes.bfloat16

DT = 0.01
B, T, IN, ST, NL, OUT, H = 2048, 32, 16, 16, 128, 8, 64
NCORES = 8
BL = B // NCORES      # 256
NR = T // 2           # 16 rounds
NG = T // 4           # 8 MLP groups
COLD = int(os.environ.get("K_COLD", "3"))
MU = float(os.environ.get("K_MU", "0.7"))
T_STEPS = T           # test.py compatibility
BB_B3 = 0.0


# ---------------------------------------------------------------------------
# host-side folding (identical math to v1) + v2 matrices
# ---------------------------------------------------------------------------

def _dadd(*ds):
    out = {}
    for d in ds:
        for k, v in d.items():
            out[k] = out.get(k, 0) + v
    return out


def _dmul(d, M):
    return {k: v @ M for k, v in d.items()}


def _dscale(d, s):
    return {k: s * v for k, v in d.items()}


def fold_matrices(inp):
    f64 = lambda k: np.asarray(inp[k], np.float64)
    A_T, Bw_T, By_T = f64("A_T"), f64("Bw_T"), f64("By_T")
    Cv_T, Dvw_T, Dvy_T = f64("Cv_T"), f64("Dvw_T"), f64("Dvy_T")
    Cu_T, Duw_T, Duy_T = f64("Cu_T"), f64("Duw_T"), f64("Duy_T")
    I16 = np.eye(16)
    Z16 = np.zeros((16, 16))
    X = {"XY": np.vstack([I16, Z16])}
    Y = {"XY": np.vstack([Z16, I16])}

    def K_of(Xd, s):
        return _dadd(_dmul(Xd, A_T), _dmul(Y, By_T), {f"W{s}": Bw_T})

    K1 = K_of(X, 1)
    X2 = _dadd(X, _dscale(K1, DT / 2))
    K2 = K_of(X2, 2)
    X3 = _dadd(X, _dscale(K2, DT / 2))
    K3 = K_of(X3, 3)
    X4 = _dadd(X, _dscale(K3, DT))
    K4 = K_of(X4, 4)
    XND = _dscale(_dadd(K1, _dscale(K2, 2), _dscale(K3, 2), K4), DT / 6)
    C1 = _dadd(_dmul(X, Cv_T), _dmul(Y, Dvy_T))["XY"]
    U = _dadd({"XY": np.vstack([Cu_T, Duy_T])}, {"W1": Duw_T})
    return {"C1": C1, "XND": XND, "U": U, "Dvw": Dvw_T}


def pad48(M):
    M = np.asarray(M)
    out = np.zeros((48, M.shape[1]))
    out[0:ST] = M[0:ST]
    out[32:48] = M[ST:2 * ST]
    return out


def build_mats(inp):
    mats = fold_matrices(inp)
    Dvw64 = mats["Dvw"]
    XND_W64 = sum(mats["XND"][f"W{s}"] for s in range(1, 5))
    Mc = (MU * Dvw64) @ np.linalg.inv(np.eye(NL) - MU * Dvw64)
    UW2 = (np.eye(NL) + Mc) @ mats["U"]["W1"]
    UW3n = -(Mc @ mats["U"]["W1"])
    XNDU = np.zeros((48, 48))
    XNDU[:, 0:8] = pad48(mats["U"]["XY"])
    XNDU[:, 32:48] = pad48(mats["XND"]["XY"])
    W1 = np.asarray(inp["W1"], np.float64)
    W2 = np.asarray(inp["W2"], np.float64)
    W3 = np.asarray(inp["W3"], np.float64)
    W1blk = np.zeros((32, 128))
    W1blk[0:16, 0:64] = W1
    W1blk[16:32, 64:128] = W1
    W2blk = np.zeros((128, 128))
    W2blk[0:64, 0:64] = W2
    W2blk[64:128, 64:128] = W2
    W3stk = np.zeros((128, 2))
    W3stk[0:64, 0:1] = W3
    W3stk[64:128, 1:2] = W3
    return {
        "C1XY": pad48(mats["C1"]),          # [48,128]
        "Dvw": Dvw64,                        # [128,128]
        "XNDU": XNDU,                        # [48,48]
        "XND_W": XND_W64,                    # [128,16]
        "UW2": UW2,                          # [128,8]
        "UW3n": UW3n,                        # [128,8]
        "W1blk": W1blk, "W2blk": W2blk, "W3stk": W3stk,
    }


def pack_blob(mats):
    entries = ["C1XY", "Dvw", "XNDU", "XND_W", "UW2", "UW3n",
               "W1blk", "W2blk", "W3stk"]
    col = 0
    offsets = {}
    cols_total = sum(int(np.asarray(mats[e]).shape[1]) for e in entries)
    blob = np.zeros((128, cols_total), np.float32)
    for name in entries:
        M = np.asarray(mats[name])
        k, m = M.shape
        blob[0:k, col:col + m] = M.astype(np.float32)
        offsets[name] = (0, k, m, col)
        col += m
    return blob.astype(bf16), offsets


# ---------------------------------------------------------------------------
# numpy emulator of the exact kernel dataflow
# ---------------------------------------------------------------------------

def emulate(inp, t_steps=None):
    t_steps = t_steps or T
    r = lambda a: a.astype(bf16).astype(np.float32)
    mats = fold_matrices(inp)
    m2 = build_mats(inp)
    C1 = r(mats["C1"].astype(np.float32))
    XND_XY = r(pad48(mats["XND"]["XY"]).astype(np.float32))[
        np.r_[0:16, 32:48]]
    XND_W = r(m2["XND_W"].astype(np.float32))
    U_XY = r(pad48(mats["U"]["XY"]).astype(np.float32))[np.r_[0:16, 32:48]]
    UW2 = r(m2["UW2"].astype(np.float32))
    UW3n = r(m2["UW3n"].astype(np.float32))
    Dvw = r(mats["Dvw"].astype(np.float32))
    obs = np.asarray(inp["obs"], np.float32)
    ys = [r(obs[:, t, :]) for t in range(t_steps)]
    xs = {0: r(np.asarray(inp["x0"], np.float32))}
    means = np.zeros((B, t_steps, OUT), np.float32)
    xy0 = np.hstack([xs[0], ys[0]])
    w = np.zeros((B, NL), np.float32)
    for i in range(COLD):
        w = r(np.tanh(xy0 @ C1 + w @ Dvw))
    i0 = {0: w}
    i0[1] = r(np.tanh(xs[0] @ C1[:ST] + ys[1] @ C1[ST:] + i0[0] @ Dvw))
    i1, i2, z1 = {}, {}, {}
    for rr in range(t_steps // 2 + 1):
        t = 2 * rr
        for j in (0, 1):      # x chain
            tt = t + j
            if tt >= t_steps:
                break
            xy = np.hstack([xs[tt], ys[tt]])
            xs[tt + 1] = r(xs[tt] + xy @ XND_XY + i0[tt] @ XND_W)
        for j in (0, 1):      # refine 1
            tt = t + j
            if tt >= t_steps:
                break
            xy = np.hstack([xs[tt], ys[tt]])
            z1[tt] = xy @ C1 + i0[tt] @ Dvw
            i1[tt] = r(np.tanh(z1[tt]))
        for j in (0, 1):      # refine 2 of previous round's steps + u
            tt = t - 2 + j
            if tt < 0 or tt >= t_steps:
                continue
            xy = np.hstack([xs[tt], ys[tt]])
            i2[tt] = r(np.tanh(z1[tt] + r(i1[tt] - i0[tt]) @ Dvw))
            means[:, tt] = xy @ U_XY + i2[tt] @ UW2 + i1[tt] @ UW3n
        if t + 2 < t_steps:   # i0 wavefront columns (stale x, warm-diag)
            xcol = xs[max(0, t - 2)]
            for j in (0, 1):
                tp = t + 2 + j
                if tp >= t_steps:
                    break
                z = xcol @ C1[:ST] + ys[tp] @ C1[ST:] + i0[t + j] @ Dvw
                i0[tp] = r(np.tanh(z))
    W1m, W2m, W3m = (r(np.asarray(inp[k], np.float32))
                     for k in ("W1", "W2", "W3"))
    b1, b2, b3 = (np.asarray(inp[k], np.float32)
                  for k in ("b1", "b2", "b3"))
    of = r(obs.reshape(-1, IN))
    h = r(np.tanh(of @ W1m + b1))
    h = r(np.tanh(h @ W2m + b2))
    v = (h @ W3m + b3).reshape(B, T, 1)
    ls = np.broadcast_to(np.asarray(inp["log_stds"], np.float32),
                         means.shape)
    return np.concatenate([means, ls, v], -1)


# ---------------------------------------------------------------------------
# Bass program
# ---------------------------------------------------------------------------

def build_program(offsets, t_steps=None):
    import concourse.bacc as bacc
    import concourse.mybir as mybir
    from concourse import tile

    f32 = mybir.dt.float32
    bf = mybir.dt.bfloat16
    Tanh = mybir.ActivationFunctionType.Tanh
    Mul = mybir.AluOpType.mult
    Add = mybir.AluOpType.add

    nc = bacc.Bacc("TRN2", target_bir_lowering=False, debug=False,
                   num_devices=NCORES)

    cols_total = max(c + m for (_, _, m, c) in offsets.values())
    yin_d = nc.dram_tensor("yin", [4, IN, 2048], bf, kind="ExternalInput")
    omlp_d = nc.dram_tensor("omlp", [32, 4096], bf, kind="ExternalInput")
    init_d = nc.dram_tensor("initd", [48, 1024], bf, kind="ExternalInput")
    zp_d = nc.dram_tensor("zpad", [16, 512 * NR], bf, kind="ExternalInput")
    wb_d = nc.dram_tensor("wblob", [128, cols_total], bf,
                          kind="ExternalInput")
    bb_d = nc.dram_tensor("bblob", [128, 2], f32, kind="ExternalInput")
    means_d = nc.dram_tensor("means_o", [NR // 2, OUT, 1024], f32,
                             kind="ExternalOutput")
    value_d = nc.dram_tensor("value_o", [NG, 2, 512], f32,
                             kind="ExternalOutput")

    with tile.TileContext(nc) as tc:
        with (
            tc.tile_pool(name="const", bufs=1) as constp,
            tc.tile_pool(name="wi0", bufs=3) as wi0p,
            tc.tile_pool(name="dt", bufs=2) as dtp,
            tc.tile_pool(name="w12", bufs=2) as w12p,
            tc.tile_pool(name="wcold", bufs=2) as wcoldp,
            tc.tile_pool(name="h", bufs=2) as hp,
            tc.tile_pool(name="mst", bufs=2) as mstp,
            tc.tile_pool(name="vst", bufs=2) as vstp,
            tc.tile_pool(name="pi0", bufs=2, space="PSUM") as pi0p,
            tc.tile_pool(name="p12", bufs=1, space="PSUM") as p12p,
            tc.tile_pool(name="sband2", bufs=1, space="PSUM") as sbp2,
            tc.tile_pool(name="sband", bufs=1, space="PSUM") as sbp,
            tc.tile_pool(name="mlpp", bufs=1, space="PSUM") as mlppp,
        ):
            # tanh table warmup overlapping the initial DMAs
            wu = constp.tile([1, 8], f32, tag="wu", name="wu")
            nc.vector.memset(wu[:], 0.0)
            wuo = constp.tile([1, 8], bf, tag="wuo", name="wuo")
            nc.scalar.activation(wuo[:], wu[:], Tanh)

            WB = constp.tile([128, cols_total], bf, tag="wb", name="WB")
            BB = constp.tile([128, 2], f32, tag="bb", name="BB")
            XYR = constp.tile([48, 512 * NR], bf, tag="xyr", name="XYR")
            OM = constp.tile([32, 4096], bf, tag="om", name="OM")

            def w_ap(name):
                row, k, m, c = offsets[name]
                return WB[row:row + k, c:c + m]

            solve_cols = offsets["W1blk"][3]
            # pad band rows 16:32 zeroed via DMA (a gpsimd memset has a
            # 32-partition alignment rule, and memsetting rows 0:32 stalls
            # the x0 DMA ~7us behind the Pool engine).  x-rows need no
            # zeroing: every slot's x-region is written before it is read.
            # Cold-critical transfers first: the HWDGE queue runs in
            # emission order, so bulk y/MLP loads go after the cold chain.
            nc.sync.dma_start(WB[:, 0:solve_cols], wb_d[:, 0:solve_cols])
            nc.sync.dma_start(XYR[0:48, 0:1024], init_d[:])

            S = sbp.tile([128, 512], f32, tag="s", name="S")
            S2 = sbp2.tile([128, 512], f32, tag="s2", name="S2")
            MP = mlppp.tile([128, 512], f32, tag="mp", name="MP")
            def bulk_dmas():
                nc.sync.dma_start(XYR[16:32, 1024:512 * NR],
                                  zp_d[:, 1024:512 * NR])
                nc.sync.dma_start(XYR[32:48, 1024:2048],
                                  yin_d[0][:, 1024:2048])
                for g4 in range(1, 4):
                    nc.sync.dma_start(XYR[32:48, 2048 * g4:2048 * (g4 + 1)],
                                      yin_d[g4])
                nc.sync.dma_start(BB[:], bb_d[:])
                nc.sync.dma_start(WB[:, solve_cols:cols_total],
                                  wb_d[:, solve_cols:cols_total])
                nc.sync.dma_start(OM[:], omlp_d[:])

            # ---- cold start: COLD iterations for i0(0), then i0(1)
            wc = None
            Wi0 = wi0p.tile([128, 512], bf, tag="wi0", name="Wi0c")
            for i in range(COLD):
                pc = pi0p.tile([128, 512], f32, tag="pi0", name=f"pc{i}")
                last = i == COLD - 1
                nc.tensor.matmul(pc[:, 0:256], w_ap("C1XY"), XYR[:, 0:256],
                                 start=True, stop=(i == 0))
                if i > 0:
                    nc.tensor.matmul(pc[:, 0:256], w_ap("Dvw"), wc[:],
                                     start=False, stop=True)
                if last:
                    nc.scalar.activation(Wi0[:, 0:256], pc[:, 0:256], Tanh)
                else:
                    wc = wcoldp.tile([128, 256], bf, tag="wc", name=f"wc{i}")
                    nc.scalar.activation(wc[:], pc[:, 0:256], Tanh)
            pc = pi0p.tile([128, 512], f32, tag="pi0", name="pc_i01")
            nc.tensor.matmul(pc[:, 0:256], w_ap("C1XY"), XYR[:, 256:512],
                             start=True, stop=False)
            nc.tensor.matmul(pc[:, 0:256], w_ap("Dvw"), Wi0[:, 0:256],
                             start=False, stop=True)
            nc.scalar.activation(Wi0[:, 256:512], pc[:, 0:256], Tanh)
            bulk_dmas()

            def uslot(k):
                us = k % 4
                return (S if us < 2 else S2), 64 * (us % 2)

            W12 = None          # previous round's T12 output
            Wi0p_prev = None    # i0pair of the previous round's steps
            h12 = h22 = None
            MSTt = None

            for rr in range(NR):
                t0 = 2 * rr
                q = rr % 2
                qn = (rr - 1) % 2
                s, sp_, sn = rr, max(rr - 1, 0), rr + 1
                cs = lambda k, a=0, b=512: slice(512 * k + a, 512 * k + b)
                last_round = rr == NR - 1
                Wi0p = Wi0            # i0(t0), i0(t0+1)
                W12p = W12

                # stale-x dup into next slot's x-region for the i0 columns:
                # x(t0-2) (slot sp, written a full round ago) keeps the
                # x-chain off the T0->T0 critical cycle; round 0 uses x(0)
                if not last_round:
                    sx = s if rr == 0 else sp_
                    nc.vector.tensor_copy(XYR[0:16, cs(sn, 0, 256)],
                                          XYR[0:16, cs(sx, 0, 256)])
                    nc.vector.tensor_copy(XYR[0:16, cs(sn, 256, 512)],
                                          XYR[0:16, cs(sx, 0, 256)])
                    # T0 critical path: i0 psum + warm Dvw + tanh, emitted
                    # first so the engine queues prioritize the cycle
                    Pi0 = pi0p.tile([128, 512], f32, tag="pi0", name="Pi0")
                    nc.tensor.matmul(Pi0[:], w_ap("C1XY"), XYR[:, cs(sn)],
                                     start=True, stop=False)
                    nc.tensor.matmul(Pi0[:], w_ap("Dvw"), Wi0p[:],
                                     start=False, stop=True)
                    Wi0 = wi0p.tile([128, 512], bf, tag="wi0", name="Wi0")
                    nc.scalar.activation(Wi0[:], Pi0[:], Tanh)

                # deferred u-finish for round rr-2's steps (parity q): its
                # UW2 input i2pair(rr-2) was computed by LAST round's T12,
                # so everything here is ready when the PE queue reaches it
                # (an in-round UW2 would stall the queue behind T12)
                if rr >= 2:
                    SU2, ub2 = uslot(rr - 2)
                    nc.tensor.matmul(SU2[ub2:ub2 + 8, 0:512],
                                     w_ap("UW2"), W12p[:, 512 * (rr % 2):
                                                       512 * (rr % 2) + 512],
                                     start=False, stop=True)
                    half = (rr - 2) % 2
                    if half == 0:
                        MSTt = mstp.tile([OUT, 1024], f32, tag="mst",
                                         name="MST")
                    nc.vector.tensor_copy(
                        MSTt[:, 512 * half:512 * half + 512],
                        SU2[ub2:ub2 + 8, 0:512])
                    if half == 1:
                        nc.sync.dma_start(means_d[(rr - 2) // 2], MSTt[:])

                # xnd(t0) + u-xy(t0)  [uslot rows: u +0:8, xnd +32:48]
                SU, ub = uslot(rr)
                nc.tensor.matmul(SU[ub:ub + 48, 0:256],
                                 w_ap("XNDU"), XYR[:, cs(s, 0, 256)],
                                 start=True, stop=False)
                nc.tensor.matmul(SU[ub + 32:ub + 48, 0:256],
                                 w_ap("XND_W"), Wi0p[:, 0:256],
                                 start=False, stop=True,
                                 tile_position=(0, ub + 32))
                # x(t0+1) = x(t0) + xnd(t0)
                nc.vector.scalar_tensor_tensor(
                    XYR[0:16, cs(s, 256, 512)], XYR[0:16, cs(s, 0, 256)],
                    1.0, SU[ub + 32:ub + 48, 0:256], Mul, Add)

                # T12 psum: i1pair | i2pair.  The i1 C1 matmul is split so
                # only the x(t0+1)-dependent half sits behind upd#1; the
                # first start=True wipes bank A full-width so every later
                # MM accumulates with start=False.
                # bank roles alternate by round parity: i1pair -> bank
                # A=512*p; i2pair accumulates Dvw@(i1-i0) ONTO the retained
                # fp32 preact of last round's i1pair in the other bank
                # (saves the C1 recompute for i2)
                p_ = rr % 2
                A_ = 512 * p_
                B_ = 512 * (1 - p_)
                P12 = p12p.tile([128, 1024], f32, tag="p12", name="P12")
                nc.tensor.matmul(P12[:, A_:A_ + 256], w_ap("C1XY"),
                                 XYR[:, cs(s, 0, 256)],
                                 start=True, stop=False)
                nc.tensor.matmul(P12[:, A_:A_ + 512], w_ap("Dvw"), Wi0p[:],
                                 start=False, stop=False)
                if rr > 0:
                    Dt = dtp.tile([128, 512], bf, tag="dt", name="Dt")
                    nc.vector.tensor_tensor(
                        Dt[:], W12p[:, B_:B_ + 512], Wi0p_prev[:],
                        mybir.AluOpType.subtract)
                    nc.tensor.matmul(P12[:, B_:B_ + 512], w_ap("Dvw"),
                                     Dt[:], start=False, stop=True)
                    SUp, ubp = uslot(rr - 1)
                    # u(rr-1 steps): UW3n part (i1pair available now)
                    nc.tensor.matmul(SUp[ubp:ubp + 8, 0:512],
                                     w_ap("UW3n"), W12p[:, B_:B_ + 512],
                                     start=False, stop=False)
                # x(t0+1)-dependent i1 half, emitted last among bank-A MMs
                nc.tensor.matmul(P12[:, A_ + 256:A_ + 512], w_ap("C1XY"),
                                 XYR[:, cs(s, 256, 512)],
                                 start=False, stop=True)

                # xnd(t1) + u-xy(t1)
                # start=False: the t0 fused MM's start=True already cleared
                # these partitions bank-wide (PSUM start clears the full
                # bank row for written partitions)
                nc.tensor.matmul(SU[ub:ub + 48, 256:512],
                                 w_ap("XNDU"), XYR[:, cs(s, 256, 512)],
                                 start=False, stop=False)
                if not last_round:
                    nc.tensor.matmul(SU[ub + 32:ub + 48, 256:512],
                                     w_ap("XND_W"), Wi0p[:, 256:512],
                                     start=False, stop=True,
                                     tile_position=(0, ub + 32))
                    # x(t0+2) = x(t0+1) + xnd(t1); overwrites the stale-x
                    # dup AFTER this round's Pi0 C1 matmul read it (WAR)
                    nc.vector.scalar_tensor_tensor(
                        XYR[0:16, cs(sn, 0, 256)],
                        XYR[0:16, cs(s, 256, 512)], 1.0,
                        SU[ub + 32:ub + 48, 256:512], Mul, Add)

                # T12 tanh
                W12 = w12p.tile([128, 1024], bf, tag="w12", name="W12")
                if rr > 0:
                    nc.scalar.activation(W12[:], P12[:], Tanh)
                else:
                    nc.scalar.activation(W12[:, 0:512], P12[:, 0:512], Tanh)

                # value MLP
                g = rr // 2
                Wi0p_prev = Wi0p
                if rr % 2 == 0:
                    nc.tensor.matmul(MP[:], w_ap("W1blk"),
                                     OM[:, 512 * g:512 * g + 512],
                                     start=True, stop=True)
                    h12 = hp.tile([128, 512], bf, tag="h", name="h12")
                    nc.scalar.activation(h12[:], MP[:], Tanh,
                                         bias=BB[:, 0:1])
                else:
                    nc.tensor.matmul(MP[:], w_ap("W2blk"), h12[:],
                                     start=True, stop=True)
                    h22 = hp.tile([128, 512], bf, tag="h", name="h22")
                    nc.scalar.activation(h22[:], MP[:], Tanh,
                                         bias=BB[:, 1:2])
                    nc.tensor.matmul(MP[0:2, :], w_ap("W3stk"), h22[:],
                                     start=True, stop=True)
                    VST = vstp.tile([2, 512], f32, tag="vst", name="VST")
                    nc.vector.tensor_copy(VST[:], MP[0:2, :])
                    nc.sync.dma_start(value_d[g], VST[:])

            # ---- epilogue: u(14)-finish, then i2 + u for round 15's steps
            q15 = (NR - 1) % 2
            q14 = (NR - 2) % 2
            s15 = NR - 1
            SU14 = S2
            nc.tensor.matmul(SU14[0:8, 0:512],
                             w_ap("UW2"), W12[:, 0:512],
                             start=False, stop=True)
            MSTt = mstp.tile([OUT, 1024], f32, tag="mst", name="MSTe")
            nc.vector.tensor_copy(MSTt[:, 0:512], SU14[0:8, 0:512])
            P12e = p12p.tile([128, 1024], f32, tag="p12", name="P12e")
            nc.tensor.matmul(P12e[:, 0:512], w_ap("C1XY"),
                             XYR[:, 512 * s15:512 * s15 + 512],
                             start=True, stop=False)
            nc.tensor.matmul(P12e[:, 0:512], w_ap("Dvw"),
                             W12[:, 512:1024], start=False, stop=True)
            nc.tensor.matmul(S2[64:72, 0:512],
                             w_ap("UW3n"), W12[:, 512:1024],
                             start=False, stop=False)
            Wi2e = wi0p.tile([128, 512], bf, tag="wi0", name="Wi2e")
            nc.scalar.activation(Wi2e[:], P12e[:, 0:512], Tanh)
            nc.tensor.matmul(S2[64:72, 0:512],
                             w_ap("UW2"), Wi2e[:], start=False, stop=True)
            nc.vector.tensor_copy(MSTt[:, 512:1024], S2[64:72, 0:512])
            nc.sync.dma_start(means_d[NR // 2 - 1], MSTt[:])
    nc.compile()
    return nc


# ---------------------------------------------------------------------------
# host glue
# ---------------------------------------------------------------------------

def _prep_inputs(inputs):
    obs = np.asarray(inputs["obs"], np.float32)
    x0 = np.asarray(inputs["x0"], np.float32)
    mats = build_mats(inputs)
    blob, offsets = pack_blob(mats)
    bb = np.zeros((128, 2), np.float32)
    b1 = np.asarray(inputs["b1"], np.float32)
    b2 = np.asarray(inputs["b2"], np.float32)
    bb[0:H, 0] = bb[H:2 * H, 0] = b1
    bb[0:H, 1] = bb[H:2 * H, 1] = b2

    in_maps = []
    for m in range(NCORES):
        osh = obs[m * BL:(m + 1) * BL]            # [BL, T, IN]
        obs_t = np.ascontiguousarray(osh.transpose(1, 2, 0))  # [T, IN, BL]
        yin = obs_t.reshape(NR, 2, IN, BL).transpose(0, 2, 1, 3) \
            .reshape(NR, IN, 512).astype(bf16)
        yin4 = np.ascontiguousarray(
            yin.reshape(4, 4, IN, 512).transpose(0, 2, 1, 3)
            .reshape(4, IN, 2048))
        omlp = np.zeros((32, 4096), np.float32)
        for g in range(NG):
            omlp[0:16, 512 * g:512 * (g + 1)] = yin[2 * g].astype(np.float32)
            omlp[16:32, 512 * g:512 * (g + 1)] = \
                yin[2 * g + 1].astype(np.float32)
        x0t = np.ascontiguousarray(x0[m * BL:(m + 1) * BL].T)  # [16,256]
        init = np.zeros((48, 1024), np.float32)
        init[0:16, 0:256] = x0t
        init[0:16, 256:512] = x0t
        init[32:48, :] = yin4[0][:, 0:1024].astype(np.float32)
        in_maps.append({
            "yin": yin4,
            "zpad": np.zeros((16, 512 * NR), bf16),
            "omlp": omlp.astype(bf16),
            "initd": init.astype(bf16), "wblob": blob, "bblob": bb,
        })
    return in_maps, offsets


def assemble(inputs, results):
    means = np.zeros((B, T, OUT), np.float32)
    value = np.zeros((B, T, 1), np.float32)
    b3 = float(np.asarray(inputs["b3"], np.float32).ravel()[0])
    for m, r in enumerate(results):
        md = r["means_o"]            # [8, OUT, 1024]
        md = md.reshape(NR // 2, OUT, 2, 2, BL)
        # axes (k, o, h, j, b) -> t = 4k + 2h + j
        means[m * BL:(m + 1) * BL] = md.transpose(4, 0, 2, 3, 1) \
            .reshape(BL, T, OUT)
        vd = r["value_o"].reshape(NG, 2, 2, BL)   # (g, blk, dt, b)
        value[m * BL:(m + 1) * BL, :, 0] = \
            vd.transpose(3, 0, 1, 2).reshape(BL, T) + b3
    ls = np.broadcast_to(
        np.asarray(inputs["log_stds"], np.float32), means.shape)
    return np.concatenate([means, ls, value], -1)


def run(inputs, t_steps=None, trace=False):
    from concourse.bass_utils import run_bass_kernel_spmd
    in_maps, offsets = _prep_inputs(inputs)
    nc = build_program(offsets)
    res = run_bass_kernel_spmd(nc, in_maps, list(range(NCORES)),
                               trace=trace)
    return res


def kernel(**inputs):
    res = run(inputs)
    return assemble(inputs, res.results)


if __name__ == "__main__":
    pass
